# revision 18
# baseline (speedup 1.0000x reference)
"""Quantum angle-encoder state-vector kernel for Trainium2 (8 NeuronCores).

Problem: for each batch element b and qubit q, the gate rz*ry applied to |0>
gives a per-qubit 2-vector
    col0 = ( cos(ry/2)*cos(rz/2), -cos(ry/2)*sin(rz/2) )   (complex)
    col1 = ( sin(ry/2)*cos(rz/2),  sin(ry/2)*sin(rz/2) )
and the output state is the Kronecker product over the 16 qubits
(qubit 0 = most significant bit), i.e. a length-65536 complex vector per b.

Kernel strategy (pure data parallel over batch, 32 batch rows per core):
  * v = v_hi (x) v_lo, each a length-256 Kronecker product of 8 qubits.
    Both are built with log-doubling on the VectorEngine (batch rows on
    SBUF partitions, state index on the free axis).
  * The 256x256 outer product v_hi (x) v_lo is a K=2 matmul on the
    TensorEngine:  out[i, n] = hi_r[i]*rhs0[n] + hi_i[i]*rhs1[n]
    with rhs columns pre-interleaved so PSUM comes out directly in
    numpy complex64 memory layout (re, im pairs).
  * PSUM -> SBUF copy (ScalarE/VectorE alternating), SBUF -> HBM DMA.
Output per core: [32, 2, 128, 512] f32 == [32, 65536] complex64.
"""

import numpy as np

import concourse.bass as bass
import concourse.mybir as mybir
import concourse.tile as tile
from concourse.bass_utils import run_bass_kernel_spmd

N_CORES = 8
B, Q = 256, 16
BC = B // N_CORES  # batch rows per core
HQ = Q // 2  # qubits per half
HL = 1 << HQ  # 256: length of each half-product
F32 = mybir.dt.float32
BF16 = mybir.dt.bfloat16
I32 = mybir.dt.int32
PI_HALF = float(np.pi / 2)

_AF = mybir.ActivationFunctionType
_OP = mybir.AluOpType


def _emit_chain_polar(nc, pool, MAG0, PH0, MAG1, PH1, pih):
    """Build BOTH 8-qubit Kronecker half-products in polar form.

    Per-qubit column entries are (magnitude, phase); a Kronecker step is then
    just magnitude-multiply (ACT) and phase-add (DVE) with per-partition
    scalars -- two independent single-engine chains. Scalar tiles are
    [2*BC, HQ] stacked (rows 0..BC-1 = qubits 0..7, rows BC.. = 8..15).
    Converts to cartesian at the end: one cos/sin pair on [2*BC, HL].

    Returns (vr, vi) tiles [2*BC, HL]: top rows = hi product, bottom = lo.
    """
    P2 = 2 * BC
    mA = pool.tile([P2, HL], F32, tag="st_mA")
    mB = pool.tile([P2, HL], F32, tag="st_mB")
    tA = pool.tile([P2, HL], F32, tag="st_tA")
    tB = pool.tile([P2, HL], F32, tag="st_tB")

    q = HQ - 1
    nc.scalar.copy(mA[:, 0:1], MAG0[:, q : q + 1])
    nc.scalar.copy(mA[:, 1:2], MAG1[:, q : q + 1])
    nc.vector.tensor_copy(tA[:, 0:1], PH0[:, q : q + 1])
    nc.vector.tensor_copy(tA[:, 1:2], PH1[:, q : q + 1])

    cur_m, cur_t, nxt_m, nxt_t = mA, tA, mB, tB
    L = 2
    for q in range(HQ - 2, -1, -1):
        for t, (MG, PH) in enumerate(((MAG0, PH0), (MAG1, PH1))):
            lo, hi = t * L, (t + 1) * L
            nc.scalar.mul(nxt_m[:, lo:hi], cur_m[:, 0:L], MG[:, q : q + 1])
            nc.vector.tensor_scalar_add(nxt_t[:, lo:hi], cur_t[:, 0:L], PH[:, q : q + 1])
        cur_m, nxt_m = nxt_m, cur_m
        cur_t, nxt_t = nxt_t, cur_t
        L *= 2

    # Range-reduce the accumulated phase into [-pi, pi] (the Sin LUT is
    # only accurate there): theta -= 2*pi*round(theta/(2*pi)), with the
    # round done by an f32->int32->f32 cast round-trip and 2*pi applied in
    # two fp32 terms for accuracy.
    INV2PI = float(1.0 / (2.0 * np.pi))
    TWO_PI_HI = float(np.float32(2.0 * np.pi))
    TWO_PI_LO = float(2.0 * np.pi - float(np.float32(2.0 * np.pi)))
    t1 = pool.tile([P2, HL], F32, tag="st_rr1")
    nc.vector.tensor_scalar_mul(t1[:], cur_t[:], INV2PI)
    ni = pool.tile([P2, HL], I32, tag="st_ni")
    nc.vector.tensor_copy(ni[:], t1[:])
    nf = pool.tile([P2, HL], F32, tag="st_nf")
    nc.vector.tensor_copy(nf[:], ni[:])
    r1 = pool.tile([P2, HL], F32, tag="st_rr2")
    nc.vector.scalar_tensor_tensor(
        r1[:], nf[:], -TWO_PI_HI, cur_t[:], op0=_OP.mult, op1=_OP.add
    )
    red = pool.tile([P2, HL], F32, tag="st_red")
    nc.vector.scalar_tensor_tensor(
        red[:], nf[:], -TWO_PI_LO, r1[:], op0=_OP.mult, op1=_OP.add
    )

    # cos(theta) = sin(theta + pi/2) needs its own reduction -- adding pi/2
    # to the reduced sin argument would leave [-pi, pi] again.
    thc = pool.tile([P2, HL], F32, tag="st_thc")
    nc.vector.tensor_scalar_add(thc[:], cur_t[:], PI_HALF)
    t1c = pool.tile([P2, HL], F32, tag="st_rc1")
    nc.vector.tensor_scalar_mul(t1c[:], thc[:], INV2PI)
    nic = pool.tile([P2, HL], I32, tag="st_nic")
    nc.vector.tensor_copy(nic[:], t1c[:])
    nfc = pool.tile([P2, HL], F32, tag="st_nfc")
    nc.vector.tensor_copy(nfc[:], nic[:])
    r1c = pool.tile([P2, HL], F32, tag="st_rc2")
    nc.vector.scalar_tensor_tensor(
        r1c[:], nfc[:], -TWO_PI_HI, thc[:], op0=_OP.mult, op1=_OP.add
    )
    redc = pool.tile([P2, HL], F32, tag="st_redc")
    nc.vector.scalar_tensor_tensor(
        redc[:], nfc[:], -TWO_PI_LO, r1c[:], op0=_OP.mult, op1=_OP.add
    )

    cosb = pool.tile([P2, HL], F32, tag="st_cos")
    sinb = pool.tile([P2, HL], F32, tag="st_sin")
    nc.scalar.activation(cosb[:], redc[:], _AF.Sin, scale=1.0)
    nc.scalar.activation(sinb[:], red[:], _AF.Sin, scale=1.0)
    vr = pool.tile([P2, HL], F32, tag="st_vr")
    vi = pool.tile([P2, HL], F32, tag="st_vi")
    nc.vector.tensor_mul(vr[:], cur_m[:], cosb[:])
    nc.vector.tensor_mul(vi[:], cur_m[:], sinb[:])
    return vr, vi


def _legalize_single_wait(nc):
    """This walrus build encodes at most one semaphore wait per instruction
    ("Too many sync wait commands" otherwise). Hoist extra waits into
    standalone EventSemaphore instructions placed immediately before — a
    sequencer-level wait gates everything after it on the same engine, so
    semantics are preserved (slightly stronger ordering)."""
    cnt = 0
    for fn in nc.m.functions:
        for blk in fn.blocks:
            out = []
            for ins in blk.instructions:
                si = ins.sync_info
                if si is not None and si.on_wait is not None and len(si.on_wait) > 1:
                    waits = list(si.on_wait)
                    for w in waits[:-1]:
                        cnt += 1
                        ev = mybir.InstEventSemaphore(
                            name=f"{ins.name}-presync-{cnt}", ins=[], outs=[]
                        )
                        ev.engine = ins.engine
                        ev.sync_info = mybir.SyncInfo(on_wait=[w], on_update=[])
                        out.append(ev)
                    ins.sync_info = mybir.SyncInfo(
                        on_wait=[waits[-1]], on_update=list(si.on_update)
                    )
                out.append(ins)
            try:
                blk.instructions = out
            except Exception:
                blk.instructions[:] = out
    return cnt


def build_bass():
    nc = bass.Bass()
    ry_d = nc.dram_tensor("ry", [BC, Q], F32, kind="ExternalInput")
    rz_d = nc.dram_tensor("rz", [BC, Q], F32, kind="ExternalInput")
    out_d = nc.dram_tensor("out", [BC, 2, 128, 512], F32, kind="ExternalOutput")

    with tile.TileContext(nc) as tc:
        with (
            tc.tile_pool(name="io", bufs=1) as io,
            tc.tile_pool(name="stage", bufs=16) as stage,
            tc.tile_pool(name="psum", bufs=8, space="PSUM") as psum,
        ):
            P2 = 2 * BC
            # Stacked angle layout [2*BC, HQ]: rows 0..BC-1 = qubits 0..7,
            # rows BC.. = qubits 8..15 (same batch rows), so the hi and lo
            # half-products advance in ONE chain over 64 partitions.
            sry = io.tile([P2, HQ], F32, tag="sry")
            srz = io.tile([P2, HQ], F32, tag="srz")
            nc.sync.dma_start(sry[0:BC, :], ry_d[:, 0:HQ])
            nc.scalar.dma_start(sry[BC:P2, :], ry_d[:, HQ:Q])
            nc.sync.dma_start(srz[0:BC, :], rz_d[:, 0:HQ])
            nc.scalar.dma_start(srz[BC:P2, :], rz_d[:, HQ:Q])

            # Per-qubit columns in polar form:
            #   col0 = cos(ry/2) * e^{-i rz/2} -> mag |cos(ry/2)|,
            #          phase -rz/2 + pi*[cos(ry/2) < 0]
            #   col1 = sin(ry/2) * e^{+i rz/2} -> mag |sin(ry/2)|,
            #          phase +rz/2 + pi*[sin(ry/2) < 0]
            pih = io.tile([P2, 1], F32, tag="pih")
            nc.vector.memset(pih[:], PI_HALF)
            c = io.tile([P2, HQ], F32, tag="c")
            s = io.tile([P2, HQ], F32, tag="s")
            nc.scalar.activation(c[:], sry[:], _AF.Sin, bias=pih[:], scale=0.5)
            nc.scalar.activation(s[:], sry[:], _AF.Sin, scale=0.5)
            MAG0 = io.tile([P2, HQ], F32, tag="MAG0")
            MAG1 = io.tile([P2, HQ], F32, tag="MAG1")
            nc.scalar.activation(MAG0[:], c[:], _AF.Abs)
            nc.scalar.activation(MAG1[:], s[:], _AF.Abs)
            hrz = io.tile([P2, HQ], F32, tag="hrz")
            nc.vector.tensor_scalar_mul(hrz[:], srz[:], 0.5)
            mkc = io.tile([P2, HQ], F32, tag="mkc")
            mks = io.tile([P2, HQ], F32, tag="mks")
            nc.vector.tensor_scalar(mkc[:], c[:], 0.0, None, op0=_OP.is_lt)
            nc.vector.tensor_scalar(mks[:], s[:], 0.0, None, op0=_OP.is_lt)
            PH0 = io.tile([P2, HQ], F32, tag="PH0")
            PH1 = io.tile([P2, HQ], F32, tag="PH1")
            PI = float(np.pi)
            nc.vector.scalar_tensor_tensor(
                PH0[:], mkc[:], PI, hrz[:], op0=_OP.mult, op1=_OP.subtract
            )
            nc.vector.scalar_tensor_tensor(
                PH1[:], mks[:], PI, hrz[:], op0=_OP.mult, op1=_OP.add
            )

            st_r, st_i = _emit_chain_polar(nc, io, MAG0, PH0, MAG1, PH1, pih)

            # fp32 matmul on PE runs at quarter rate; instead split each fp32
            # factor into 3 bf16 terms (h + m + l covers the full 24-bit
            # mantissa) and run full-rate bf16 matmuls with K=12. Products
            # (h,h),(h,m),(m,h),(h,l),(l,h),(m,m) are kept; dropped terms are
            # <= 2^-24 relative.
            def split3(x, pfx):
                parts = []
                cur = x
                for lvl in range(3):
                    pb = io.tile([P2, HL], BF16, tag=f"{pfx}_b{lvl}")
                    nc.scalar.copy(pb[:], cur[:])
                    parts.append(pb)
                    if lvl < 2:
                        res = io.tile([P2, HL], F32, tag=f"{pfx}_r{lvl}")
                        nc.vector.tensor_sub(res[:], cur[:], pb[:])
                        cur = res
                return parts  # [h, m, l] bf16 tiles, stacked hi|lo

            r_sp = split3(st_r, "rsp")
            i_sp = split3(st_i, "isp")
            # Views: top rows = hi-half splits, bottom rows = lo-half splits.
            hr = [p[0:BC] for p in r_sp]
            hh = [p[0:BC] for p in i_sp]
            lr = [p[BC:P2] for p in r_sp]
            ll = [p[BC:P2] for p in i_sp]
            # Negated lo-imag splits; compute in the bottom partition group so
            # DVE in/out partition bases match.
            nll = []
            for lvl in range(3):
                t = io.tile([P2, HL], BF16, tag=f"nll_b{lvl}")
                nc.vector.tensor_scalar_mul(t[BC:P2, :], i_sp[lvl][BC:P2, :], -1.0)
                nll.append(t[BC:P2])

            # Term pairing (a, b): lhsT row holds hi-part a, rhs row holds
            # lo-part b. Same lhsT rows serve real (even cols) and imag (odd).
            PAIRS = [(0, 0), (0, 1), (1, 0), (0, 2), (2, 0), (1, 1)]
            K = 2 * len(PAIRS)  # 12

            # lhsT rows, flattened batch-major: rows 0..5 = hr[a_k], 6..11 = hh[a_k]
            LH = io.tile([K, BC * HL], BF16, tag="LH")
            dma_engs = [nc.sync, nc.scalar, nc.gpsimd]
            for k, (a, _) in enumerate(PAIRS):
                dma_engs[k % 3].dma_start(LH[k : k + 1, :], hr[a])
                dma_engs[(k + 1) % 3].dma_start(LH[6 + k : 7 + k, :], hh[a])

            # Interleaved rhs patterns, built batch-on-partitions then
            # flattened. PT1[b] = interleave(lr_b, ll_b)  (rows 0..5),
            # PT2[b] = interleave(-ll_b, lr_b)            (rows 6..11).
            # Built in the bottom partition group (rows BC..) so DVE in/out
            # partition bases match the lo-half source views.
            PT1 = []
            PT2 = []
            for lvl in range(3):
                t1 = io.tile([P2, 2 * HL], BF16, tag=f"PT1_{lvl}")
                v1 = t1[BC:P2, :].rearrange("p (j t) -> p j t", t=2)
                nc.vector.tensor_copy(v1[:, :, 0], lr[lvl])
                nc.vector.tensor_copy(v1[:, :, 1], ll[lvl])
                PT1.append(t1[BC:P2, :])
                t2 = io.tile([P2, 2 * HL], BF16, tag=f"PT2_{lvl}")
                v2 = t2[BC:P2, :].rearrange("p (j t) -> p j t", t=2)
                nc.vector.tensor_copy(v2[:, :, 0], nll[lvl])
                nc.vector.tensor_copy(v2[:, :, 1], lr[lvl])
                PT2.append(t2[BC:P2, :])
            RH = io.tile([K, BC * 2 * HL], BF16, tag="RH")
            for k, (_, b) in enumerate(PAIRS):
                dma_engs[(k + 2) % 3].dma_start(RH[k : k + 1, :], PT1[b])
                dma_engs[k % 3].dma_start(RH[6 + k : 7 + k, :], PT2[b])

            # out[b, ck*128+p, :] = hi[b, ck*128+p] * lo[b, :] as a K=12 matmul.
            for bi in range(BC):
                for ck in range(2):
                    acc = psum.tile([128, 512], F32, tag="acc")
                    lh_off = bi * HL + ck * 128
                    rh_off = bi * 2 * HL
                    nc.tensor.matmul(
                        acc[:],
                        LH[:, lh_off : lh_off + 128],
                        RH[:, rh_off : rh_off + 2 * HL],
                        start=True,
                        stop=True,
                    )
                    ot = stage.tile([128, 512], F32, tag="ot")
                    it = bi * 2 + ck
                    if it % 8 in (0, 3, 6):
                        nc.scalar.copy(ot[:], acc[:])
                    else:
                        nc.vector.tensor_copy(ot[:], acc[:])
                    out_eng = (nc.sync, nc.scalar, nc.sync, nc.scalar)[it % 4]
                    out_eng.dma_start(out_d[bi, ck], ot[:])
    _legalize_single_wait(nc)
    return nc


_nc_cache = None


def _get_nc():
    global _nc_cache
    if _nc_cache is None:
        _nc_cache = build_bass()
    return _nc_cache


def run(ry_angles, rz_angles, trace=False):
    """Shard over 8 cores, run, gather. Returns (out [B, 2**Q] c64, results)."""
    ry = np.ascontiguousarray(np.asarray(ry_angles, dtype=np.float32))
    rz = np.ascontiguousarray(np.asarray(rz_angles, dtype=np.float32))
    assert ry.shape == (B, Q) and rz.shape == (B, Q)
    nc = _get_nc()
    in_maps = [
        {
            "ry": np.ascontiguousarray(ry[k * BC : (k + 1) * BC]),
            "rz": np.ascontiguousarray(rz[k * BC : (k + 1) * BC]),
        }
        for k in range(N_CORES)
    ]
    res = run_bass_kernel_spmd(nc, in_maps, list(range(N_CORES)), trace=trace)
    parts = [
        np.ascontiguousarray(r["out"]).reshape(BC, 2 * (1 << Q)).view(np.complex64)
        for r in res.results
    ]
    return np.concatenate(parts, axis=0), res


def kernel(ry_angles, rz_angles):
    out, _ = run(ry_angles, rz_angles, trace=False)
    return out
